# revision 1
# baseline (speedup 1.0000x reference)
"""Trainium2 Bass kernel for nn_InterpretableAttention (B=8, N=4096, DIM=1024).

Math: the reference returns softmax(q @ k^T, axis=-1)[:, 0, :] -- only row 0
of the attention matrix. So per batch b:
    q0       = Wq @ x[b,0] + bq                                  [DIM]
    v        = Wk^T @ q0                                         [DIM]
    scores_m = x[b,m] . v   (+ q0.bk, a constant -> cancels in softmax)
    out[b]   = softmax(scores)                                   [N]
bk never affects the output. The N x N score matrix and the full q/k
projections are never materialized.

Sharding: data-parallel over batch, one batch per NeuronCore (B == 8 cores),
with the tiny q0/v projection stage *tensor-parallel* over the 8 cores:
core j holds only e-chunk j of Wq^T / Wk (1 MB instead of 8 MB), computes
q0-chunk and a partial V for ALL batches, and a ReduceScatter (b-major
layout) both sums the partials and hands each core exactly its own batch's
v. Host-side resharding sends x[b] TRANSPOSED ([DIM, N]) so the big matvec
contracts over the partition axis on the tensor engine.

Per-core device pipeline (all f32):
  A) q0 chunk: 8 accumulating [128,128]x[128,8] matmuls from Wq^T tiles;
     bias add on DVE; partial V: 8 [128,128]x[128,8] matmuls from Wk rows.
     DMA to DRAM scratch, ReduceScatter(add) over all 8 cores, DMA back.
  B) scores: for each of 8 m-tiles, 8 accumulating [128,1]^T x [128,512]
     matmuls (contraction over d) -> PSUM [1,512]; per-tile max on DVE,
     PSUM->SBUF copy on ACT.
  C) softmax over [1,4096]: global max (DVE), exp with bias=-max and fused
     free-dim sum (ACT accum_out), reciprocal (DVE), scale (ACT), DMA out.
"""

import os
from contextlib import ExitStack

import numpy as np

import concourse.bass as bass  # noqa: F401
import concourse.tile as tile
from concourse import bacc, mybir
from concourse.bass_utils import run_bass_kernel_spmd

B, N, DIM = 8, 4096, 1024
P = 128          # partitions
KC = DIM // P    # 8 chunks along d (or e)
MT = 512         # m-tile (matmul moving free dim)
NMT = N // MT    # 8 m-tiles
F32 = mybir.dt.float32
MM_DT = mybir.dt.float32r if os.environ.get("KERNEL_MM_F32R", "0") == "1" else F32
COLLECTIVE = os.environ.get("KERNEL_COLLECTIVE", "0") == "1"

_program_cache = {}


def _build_program():
    key = (str(MM_DT), COLLECTIVE)
    if key in _program_cache:
        return _program_cache[key]

    nc = bacc.Bacc(
        "TRN2",
        target_bir_lowering=False,
        debug=False,
        enable_asserts=False,
        num_devices=B,
    )
    xt = nc.dram_tensor("xt", [DIM, N], F32, kind="ExternalInput").ap()
    out = nc.dram_tensor("out", [1, N], F32, kind="ExternalOutput").ap()
    if COLLECTIVE:
        # per-core slices: own e-chunk of Wq^T columns / Wk rows, all batches' x0
        wqtc = nc.dram_tensor("wqtc", [DIM, P], F32, kind="ExternalInput").ap()
        wkc = nc.dram_tensor("wkc", [P, DIM], F32, kind="ExternalInput").ap()
        x0all = nc.dram_tensor("x0all", [DIM, B], F32, kind="ExternalInput").ap()
        bqc = nc.dram_tensor("bqc", [P, 1], F32, kind="ExternalInput").ap()
        vscr_in = nc.dram_tensor("vscr_in", [B, KC, P], F32).ap()
        vscr_out = nc.dram_tensor("vscr_out", [KC, P], F32).ap()
    else:
        x0t = nc.dram_tensor("x0t", [DIM, 1], F32, kind="ExternalInput").ap()
        wqt = nc.dram_tensor("wqt", [DIM, DIM], F32, kind="ExternalInput").ap()
        wk = nc.dram_tensor("wk", [DIM, DIM], F32, kind="ExternalInput").ap()
        bqs = nc.dram_tensor("bqs", [P, KC], F32, kind="ExternalInput").ap()

    with tile.TileContext(nc) as tc, ExitStack() as ctx:
        singles = ctx.enter_context(tc.tile_pool(name="singles", bufs=1))
        xpool = ctx.enter_context(tc.tile_pool(name="xpool", bufs=16))
        pspool = ctx.enter_context(tc.tile_pool(name="pspool", bufs=2, space="PSUM"))
        pscore = ctx.enter_context(tc.tile_pool(name="pscore", bufs=4, space="PSUM"))

        # ---------------- Phase A: q0 and v (small stage) ----------------
        if COLLECTIVE:
            x0s = singles.tile([P, KC, B], F32)  # [p, d-chunk, b]
            nc.sync.dma_start(x0s, x0all.rearrange("(i p) b -> p i b", p=P))
            bqs_t = singles.tile([P, 1], F32)
            nc.sync.dma_start(bqs_t, bqc)
            wqts = singles.tile([P, KC, P], F32)  # [p(d), d-chunk, e-in-chunk]
            nc.sync.dma_start(wqts, wqtc.rearrange("(i p) e -> p i e", p=P))
            wks = singles.tile([P, DIM], F32)  # [p(e-in-chunk), d]
            nc.sync.dma_start(wks, wkc)

            # q0 own e-chunk, all batches: [128(e), 8(b)]
            q0p = pspool.tile([P, B], F32)
            for i in range(KC):
                nc.tensor.matmul(
                    q0p,
                    wqts[:, i, :],
                    x0s[:, i, :],
                    start=(i == 0),
                    stop=(i == KC - 1),
                )
            q0s = singles.tile([P, B], F32)
            nc.vector.tensor_scalar_add(q0s, q0p, bqs_t)

            # partial V^T for all batches: [128(d-in-chunk), d-chunk, b]
            vpp = pspool.tile([P, KC, B], F32)
            for k in range(KC):
                nc.tensor.matmul(
                    vpp[:, k, :],
                    wks[:, k * P : (k + 1) * P],
                    q0s,
                    start=True,
                    stop=True,
                )
            # permute free dims on the DVE copy so the DRAM DMA nests (b, k)
            # identically on both sides; b-major DRAM layout makes
            # ReduceScatter chunk r == batch r's v
            vpart = singles.tile([P, B, KC], F32)
            nc.vector.tensor_copy(vpart, vpp.rearrange("p k b -> p b k"))
            nc.sync.dma_start(vscr_in.rearrange("b k p -> p b k"), vpart)
            nc.gpsimd.collective_compute(
                "ReduceScatter",
                mybir.AluOpType.add,
                replica_groups=[list(range(B))],
                ins=[vscr_in],
                outs=[vscr_out],
            )
            vs = singles.tile([P, KC], MM_DT)
            nc.sync.dma_start(vs, vscr_out.rearrange("k p -> p k"))
        else:
            x0s = singles.tile([P, KC], F32)
            nc.sync.dma_start(x0s, x0t.rearrange("(c p) u -> p (c u)", p=P))
            bqt = singles.tile([P, KC], F32)
            nc.sync.dma_start(bqt, bqs)
            wq_all = singles.tile([P, KC, DIM], F32)
            wk_all = singles.tile([P, KC, DIM], F32)
            for i in range(KC):
                nc.sync.dma_start(wq_all[:, i, :], wqt[i * P : (i + 1) * P, :])
                nc.gpsimd.dma_start(wk_all[:, i, :], wk[i * P : (i + 1) * P, :])
            q0p = pspool.tile([P, KC], F32)
            for j in range(KC):
                for i in range(KC):
                    nc.tensor.matmul(
                        q0p[:, j : j + 1],
                        wq_all[:, i, j * P : (j + 1) * P],
                        x0s[:, i : i + 1],
                        start=(i == 0),
                        stop=(i == KC - 1),
                    )
            q0s = singles.tile([P, KC], F32)
            nc.vector.tensor_add(q0s, q0p, bqt)
            vp = pspool.tile([P, KC], F32)
            for k in range(KC):
                for j in range(KC):
                    nc.tensor.matmul(
                        vp[:, k : k + 1],
                        wk_all[:, j, k * P : (k + 1) * P],
                        q0s[:, j : j + 1],
                        start=(j == 0),
                        stop=(j == KC - 1),
                    )
            vs = singles.tile([P, KC], MM_DT)
            nc.vector.tensor_copy(vs, vp)

        # ---------------- Phase B: scores[m] = x[m] . v ----------------
        scores = singles.tile([1, N], F32)
        lmax = singles.tile([1, NMT], F32)
        dma_engines = [nc.sync, nc.gpsimd, nc.scalar]
        for t in range(NMT):
            ps = pscore.tile([1, MT], F32)
            for k in range(KC):
                xtile = xpool.tile([P, MT], MM_DT)
                if MM_DT == F32:
                    eng = dma_engines[(t * KC + k) % len(dma_engines)]
                else:
                    eng = nc.gpsimd  # only gpsimd may initiate casting DMAs
                eng.dma_start(xtile, xt[k * P : (k + 1) * P, t * MT : (t + 1) * MT])
                nc.tensor.matmul(
                    ps,
                    vs[:, k : k + 1],
                    xtile,
                    start=(k == 0),
                    stop=(k == KC - 1),
                )
            nc.vector.tensor_reduce(
                lmax[:, t : t + 1], ps, axis=mybir.AxisListType.X, op=mybir.AluOpType.max
            )
            nc.scalar.copy(scores[:, t * MT : (t + 1) * MT], ps)

        # ---------------- Phase C: softmax over [1, N] ----------------
        negmax = singles.tile([1, 1], F32)
        nc.vector.tensor_reduce(
            negmax, lmax, axis=mybir.AxisListType.X, op=mybir.AluOpType.max, negate=True
        )
        esb = singles.tile([1, N], F32)
        ssum = singles.tile([1, 1], F32)
        nc.scalar.activation(
            esb,
            scores,
            mybir.ActivationFunctionType.Exp,
            bias=negmax,
            scale=1.0,
            accum_out=ssum,
        )
        rinv = singles.tile([1, 1], F32)
        nc.vector.reciprocal(rinv, ssum)
        osb = singles.tile([1, N], F32)
        nc.scalar.activation(
            osb, esb, mybir.ActivationFunctionType.Copy, bias=0.0, scale=rinv
        )
        nc.sync.dma_start(out, osb)

    nc.compile()
    _program_cache[key] = nc
    return nc


def _make_in_maps(x, Wq, bq, Wk):
    x = np.asarray(x, dtype=np.float32)
    wq = np.asarray(Wq, np.float32)
    wk = np.asarray(Wk, np.float32)
    bq = np.asarray(bq, np.float32)
    in_maps = []
    if COLLECTIVE:
        wqt_h = np.ascontiguousarray(wq.T)  # [d, e]
        x0all_h = np.ascontiguousarray(x[:, 0, :].T)  # [d, b]
        for j in range(B):
            in_maps.append(
                {
                    "xt": np.ascontiguousarray(x[j].T),
                    "wqtc": np.ascontiguousarray(wqt_h[:, j * P : (j + 1) * P]),
                    "wkc": np.ascontiguousarray(wk[j * P : (j + 1) * P, :]),
                    "x0all": x0all_h,
                    "bqc": np.ascontiguousarray(bq[j * P : (j + 1) * P].reshape(P, 1)),
                }
            )
    else:
        wqt_h = np.ascontiguousarray(wq.T)
        bq_h = np.ascontiguousarray(bq.reshape(KC, P).T)
        for b in range(B):
            in_maps.append(
                {
                    "xt": np.ascontiguousarray(x[b].T),
                    "x0t": np.ascontiguousarray(x[b, 0].reshape(DIM, 1)),
                    "wqt": wqt_h,
                    "wk": np.ascontiguousarray(wk),
                    "bqs": bq_h,
                }
            )
    return in_maps


def kernel(x, Wq, bq, Wk, bk):
    nc = _build_program()
    in_maps = _make_in_maps(x, Wq, bq, Wk)
    res = run_bass_kernel_spmd(nc, in_maps, core_ids=list(range(B)))
    outs = [np.asarray(res.results[b]["out"]).reshape(N) for b in range(B)]
    return np.stack(outs, axis=0).astype(np.float32)



# revision 11
# speedup vs baseline: 1.5624x; 1.5624x over previous
"""Trainium2 Bass kernel for nn_InterpretableAttention (B=8, N=4096, DIM=1024).

Math: the reference returns softmax(q @ k^T, axis=-1)[:, 0, :] -- only row 0
of the attention matrix. So per batch b:
    q0       = Wq @ x[b,0] + bq                                  [DIM]
    v        = Wk^T @ q0                                         [DIM]
    scores_m = x[b,m] . v   (+ q0.bk, a constant -> cancels in softmax)
    out[b]   = softmax(scores)                                   [N]
bk never affects the output. The N x N score matrix and the full q/k
projections are never materialized.

Sharding: data-parallel over batch, one batch per NeuronCore (B == 8 cores).
Collectives on this stack cost ~75us for even a 32KB ReduceScatter (ring
algorithm, ~10us/step latency floor), so each core redundantly loads the
full Wq^T / Wk (8MB) and computes its own q0/v locally. The kernel is
HBM-DMA-bound: 16.8MB of x[b]^T plus 8.4MB of weights per core, streamed
back-to-back on both HWDGE rings (sync + scalar) so the 16 SDMA engines
never idle.

Per-core device pipeline (all f32):
  DMA   sync ring:   x0, bq, Wq^T (4MB), then x^T d-chunks 0,2,4,6 (2MB each)
        scalar ring: Wk (4MB), then x^T d-chunks 1,3,5,7
        The last two x chunks reuse the Wq/Wk SBUF slots (tag-shared pool).
  A) q0^T = x0^T Wq^T + bq as [1,1024]: 16 accumulating [128,1]^T x [128,512]
     matmuls + 2 K=1 bias matmuls; PE-transpose to [128,8].
     v^T = q0^T Wk as [1,1024]: 16 matmuls; PE-transpose to vs [128,8].
  B) scores: k-outer over d-chunks, 64 matmuls [128,1]^T x [128,512] -> 8
     PSUM accumulators [1,512] packed 4-per-bank at partitions {0,32,64,96}.
  C) softmax on [8,512]x? layout: free-axis max (DVE), cross-partition max
     (GpSimd partition_all_reduce), exp with fused row-sum (ACT accum_out),
     cross-partition sum, reciprocal, scale, one [8,512] DMA out.
"""

from contextlib import ExitStack

import numpy as np

import concourse.bass as bass  # noqa: F401
import concourse.tile as tile
from concourse import bacc, bass_isa, mybir
from concourse.bass_utils import run_bass_kernel_spmd

B, N, DIM = 8, 4096, 1024
P = 128          # partitions
KC = DIM // P    # 8 chunks along d (or e)
MT = 512         # m-tile (matmul moving free dim, PSUM f32 bank limit)
NMT = N // MT    # 8 m-tiles
F32 = mybir.dt.float32

_program_cache = {}


def _build_program():
    if "nc" in _program_cache:
        return _program_cache["nc"]

    nc = bacc.Bacc(
        "TRN2",
        target_bir_lowering=False,
        debug=False,
        enable_asserts=False,
        num_devices=B,
    )
    xt = nc.dram_tensor("xt", [DIM, N], F32, kind="ExternalInput").ap()
    wqt = nc.dram_tensor("wqt", [DIM, DIM], F32, kind="ExternalInput").ap()
    wk = nc.dram_tensor("wk", [DIM, DIM], F32, kind="ExternalInput").ap()
    x0c = nc.dram_tensor("x0c", [P, KC], F32, kind="ExternalInput").ap()
    bqr = nc.dram_tensor("bqr", [1, DIM], F32, kind="ExternalInput").ap()
    out = nc.dram_tensor("out", [2, 4 * MT], F32, kind="ExternalOutput").ap()

    with tile.TileContext(nc) as tc, ExitStack() as ctx:
        sb = ctx.enter_context(tc.tile_pool(name="sb", bufs=1))
        shared = ctx.enter_context(tc.tile_pool(name="shared", bufs=2))
        pa = ctx.enter_context(tc.tile_pool(name="pa", bufs=3, space="PSUM"))
        psc = ctx.enter_context(tc.tile_pool(name="psc", bufs=4, space="PSUM"))

        # ---------------- DMA plan ----------------
        # sync ring: small inputs, Wq^T, then even x chunks.
        # scalar ring: Wk, then odd x chunks. Rings drain round-robin on the
        # shared 16 SDMA engines, so both make ~equal progress.
        x0s = sb.tile([P, KC], F32)
        nc.sync.dma_start(x0s, x0c)
        bqs = sb.tile([1, DIM], F32)
        nc.sync.dma_start(bqs, bqr)
        # wq_all[p, i, e] = Wq^T[i*128+p, e]; wk_all[p, i, d] = Wk[i*128+p, d]
        wq_all = shared.tile([P, KC, DIM], F32, tag="w")
        nc.sync.dma_start(wq_all, wqt.rearrange("(i p) e -> p i e", p=P))
        wk_all = shared.tile([P, KC, DIM], F32, tag="w")
        nc.scalar.dma_start(wk_all, wk.rearrange("(i p) d -> p i d", p=P))
        # x chunks: xs[k][p, m] = x[b, m, k*128+p], 2MB contiguous each.
        xs = []
        for k in range(KC):
            if k < KC - 2:
                xtile = sb.tile([P, N], F32, name=f"xs{k}")
            else:
                xtile = shared.tile([P, N], F32, name=f"xs{k}", tag="w")
            eng = nc.sync if k % 2 == 0 else nc.scalar
            eng.dma_start(xtile, xt[k * P : (k + 1) * P, :])
            xs.append(xtile)

        ones = sb.tile([1, 1], F32)
        nc.gpsimd.memset(ones, 1.0)

        # ---------------- Phase A: q0 and v ----------------
        # q0^T [1, 1024] = x0^T @ Wq^T + bq, two 512-wide PSUM halves.
        q0sb = sb.tile([1, DIM], F32)
        for h in range(2):
            q0p = pa.tile([1, MT], F32, tag="ps")
            # bias first via K=1 matmul: q0p = ones^T @ bq_half
            nc.tensor.matmul(
                q0p,
                ones,
                bqs[:, h * MT : (h + 1) * MT],
                start=True,
                stop=False,
                skip_group_check=True,
            )
            for i in range(KC):
                nc.tensor.matmul(
                    q0p,
                    x0s[:, i : i + 1],
                    wq_all[:, i, h * MT : (h + 1) * MT],
                    start=False,
                    stop=(i == KC - 1),
                    skip_group_check=True,
                )
            if h == 0:
                nc.vector.tensor_copy(q0sb[:, h * MT : (h + 1) * MT], q0p)
            else:
                nc.scalar.copy(q0sb[:, h * MT : (h + 1) * MT], q0p)

        # transpose q0 -> [128, 8] (e on partitions)
        q0Tp = pa.tile([P, KC], F32, tag="ps")
        for i in range(KC):
            nc.tensor.transpose(
                q0Tp[:, i : i + 1], q0sb[:, i * P : (i + 1) * P], ones
            )
        q0T = sb.tile([P, KC], F32)
        nc.vector.tensor_copy(q0T, q0Tp)

        # v^T [1, 1024] = q0^T @ Wk
        vsb = sb.tile([1, DIM], F32)
        for h in range(2):
            vp = pa.tile([1, MT], F32, tag="ps")
            for i in range(KC):
                nc.tensor.matmul(
                    vp,
                    q0T[:, i : i + 1],
                    wk_all[:, i, h * MT : (h + 1) * MT],
                    start=(i == 0),
                    stop=(i == KC - 1),
                )
            if h == 0:
                nc.vector.tensor_copy(vsb[:, h * MT : (h + 1) * MT], vp)
            else:
                nc.scalar.copy(vsb[:, h * MT : (h + 1) * MT], vp)

        # transpose v -> vs [128, 8] (d-chunk on partitions)
        vsT = pa.tile([P, KC], F32, tag="ps")
        for i in range(KC):
            nc.tensor.transpose(
                vsT[:, i : i + 1], vsb[:, i * P : (i + 1) * P], ones
            )
        vs = sb.tile([P, KC], F32)
        nc.vector.tensor_copy(vs, vsT)

        # ---------------- Phase B: scores[m] = x[m] . v ----------------
        # 8 accumulators [1, 512], 2 per PSUM bank at partitions {0,64}.
        sc = [psc.tile([P, MT], F32, name=f"sc{i}", tag="sc") for i in range(4)]
        for k in range(KC):
            for t in range(NMT):
                bank, pos = t // 2, (t % 2) * 64
                nc.tensor.matmul(
                    sc[bank][pos : pos + 1, :],
                    vs[:, k : k + 1],
                    xs[k][:, t * MT : (t + 1) * MT],
                    start=(k == 0),
                    stop=(k == KC - 1),
                    skip_group_check=True,
                )

        # gather the 8 accumulators into rows {0, 64} of one SBUF tile:
        # sco[(t%2)*64, (t//2)*MT : +MT] = scores m-tile t. Rows other than
        # {0,64} are memset to -3e38 so they contribute exp(..)=0 downstream.
        sco = sb.tile([P, 4 * MT], F32)
        nc.vector.memset(sco, -3e38)
        for t in range(NMT):
            bank, pos = t // 2, (t % 2) * 64
            dst = sco[pos : pos + 1, bank * MT : (bank + 1) * MT]
            if t % 2 == 0:
                nc.vector.tensor_copy(dst, sc[bank][pos : pos + 1, :])
            else:
                nc.scalar.copy(dst, sc[bank][pos : pos + 1, :])

        # ---------------- Phase C: softmax (rows {0,64} are live) ----------------
        lmax = sb.tile([P, 1], F32)
        nc.vector.tensor_reduce(
            lmax, sco, axis=mybir.AxisListType.X, op=mybir.AluOpType.max
        )
        gmax = sb.tile([P, 1], F32)
        nc.gpsimd.partition_all_reduce(
            gmax, lmax, channels=P, reduce_op=bass_isa.ReduceOp.max
        )
        ngmax = sb.tile([P, 1], F32)
        nc.vector.tensor_scalar_mul(ngmax, gmax, -1.0)
        esb = sb.tile([P, 4 * MT], F32)
        ssum = sb.tile([P, 1], F32)
        nc.scalar.activation(
            esb,
            sco,
            mybir.ActivationFunctionType.Exp,
            bias=ngmax,
            scale=1.0,
            accum_out=ssum,
        )
        tsum = sb.tile([P, 1], F32)
        nc.gpsimd.partition_all_reduce(
            tsum, ssum, channels=P, reduce_op=bass_isa.ReduceOp.add
        )
        rinv = sb.tile([P, 1], F32)
        nc.vector.reciprocal(rinv, tsum)
        osb = sb.tile([P, 4 * MT], F32)
        nc.scalar.activation(
            osb, esb, mybir.ActivationFunctionType.Copy, bias=0.0, scale=rinv
        )
        # out[0] = even m-tiles (row 0), out[1] = odd m-tiles (row 64)
        nc.sync.dma_start(out[0:1, :], osb[0:1, :])
        nc.sync.dma_start(out[1:2, :], osb[64:65, :])

    nc.compile()
    _program_cache["nc"] = nc
    return nc


def _make_in_maps(x, Wq, bq, Wk):
    x = np.asarray(x, dtype=np.float32)
    wqt_h = np.ascontiguousarray(np.asarray(Wq, np.float32).T)
    wk_h = np.ascontiguousarray(np.asarray(Wk, np.float32))
    bq_h = np.asarray(bq, np.float32).reshape(1, DIM)
    in_maps = []
    for b in range(B):
        in_maps.append(
            {
                "xt": np.ascontiguousarray(x[b].T),
                "wqt": wqt_h,
                "wk": wk_h,
                "x0c": np.ascontiguousarray(x[b, 0].reshape(KC, P).T),
                "bqr": bq_h,
            }
        )
    return in_maps


def _unpack_out(arr):
    # device out is [2, 4*MT]: row r, bank c holds m-tile t = 2*c + r
    return (
        np.asarray(arr).reshape(2, NMT // 2, MT).transpose(1, 0, 2).reshape(N)
    )


def kernel(x, Wq, bq, Wk, bk):
    nc = _build_program()
    in_maps = _make_in_maps(x, Wq, bq, Wk)
    res = run_bass_kernel_spmd(nc, in_maps, core_ids=list(range(B)))
    outs = [_unpack_out(res.results[b]["out"]) for b in range(B)]
    return np.stack(outs, axis=0).astype(np.float32)
